# revision 14
# baseline (speedup 1.0000x reference)
"""Trainium2 Bass kernel for nn_Attention_57080115364834.

Reference computation (B=4, C=512, H=W=64, N=H*W=4096 tokens):
    t = x.reshape(b, c, n).swapaxes(1, 2)          # (b, n, c)
    q, k, v = t@Wq.T+bq, t@Wk.T+bk, t@Wv.T+bv
    attn = softmax(q @ k.T / sqrt(c))              # (b, n, n)
    out = (attn @ v) @ Wo.T + bo                   # (b, n, c)
    return out.reshape(b, c, h, w)                 # raw view, no permute

Sharding: 8 cores = 4 batches x 2 query-halves. Each core holds the full
x[b] (C x N, which is exactly t.T - the natural layout for Trainium
matmuls) so it computes its batch's full K^T (c,n) and VW (n,c) locally,
plus Q^T for its 2048-token half. No collectives.

Host-side algebra folds both post-attention linear steps away:
  - softmax rows sum to 1  =>  attn @ (v+bv) == attn@v + bv, so the v
    bias becomes an output bias  bo' = Wo @ bv + bo.
  - (attn @ v) @ Wo.T == attn @ (v @ Wo.T) == attn @ (t @ (Wo@Wv).T),
    so with Wvo = Wo@Wv precomputed on host, the VW projection directly
    produces final-channel values and no device-side output projection
    is needed.
The kernel returns outT (c, n) per core; the host transposes during
unsharding (a pure layout move).

Per-core dataflow (main matmuls in float32r = full-rate fp32):
  kT[c,m]   = Wk @ tC + bk   (lhsT=WkT chunk, rhs=tC chunk; bias on ACT evac)
  VW[m,c]   = tC.T @ WvoT    (lhsT=tC chunk,  rhs=WvoT)
  qT[c,n]   = Wq @ tCq + bq  per 512-token n-chunk
  ST[m,n]   = kT.T-chunks @ qT       (scores, transposed)
  P[m,n]    = exp(ST/sqrt(c))        ScalarE, no max-subtract (|scores|<~2)
  acc[m%128,n] += P                  DVE accumulate (for rowsum)
  OT[c,n]  += VW-chunk.T @ P         (PSUM-accumulated over m-tiles)
  OT[c,n]  += bo'[c-chunk] x rowsum[n]   (K=1 matmul; exact bias)
  rowsum_bc = gpsimd partition_all_reduce(acc); rinv_bc = 1/rowsum_bc
  outT[c,n] = OT * rinv_bc           (DVE, PSUM->SBUF) -> DMA
"""

import sys

for _p in ("/opt/trn_rl_repo", "/root/.axon_site/_ro/trn_rl_repo"):
    if _p not in sys.path:
        sys.path.append(_p)

import numpy as np

import concourse.bacc as bacc
import concourse.bass_isa as bass_isa
import concourse.mybir as mybir
import concourse.tile as tile
from concourse.bass_utils import run_bass_kernel_spmd

DT = mybir.dt.float32
FR = mybir.dt.float32r
AFT = mybir.ActivationFunctionType

B, C, HW = 4, 512, 4096          # batch, channels, tokens per batch
NQ = HW // 2                     # q tokens per core (2048)
CK = C // 128                    # contraction chunks (4)
MT = HW // 128                   # key/value tiles (32)
NB = NQ // 512                   # q-chunks per core (4)
SCALE = 1.0 / float(np.sqrt(C))
N_CORES = 8

_compiled = None
_ONES = np.ones(128, dtype=np.float32)


def _build():
    nc = bacc.Bacc("TRN2", target_bir_lowering=False)

    xt_e = nc.declare_dram_parameter("xt", [C, HW], FR, isOutput=False)
    xq_e = nc.declare_dram_parameter("xq", [C, NQ], FR, isOutput=False)
    wqt_e = nc.declare_dram_parameter("wqt", [C, C], FR, isOutput=False)
    wkt_e = nc.declare_dram_parameter("wkt", [C, C], FR, isOutput=False)
    wvot_e = nc.declare_dram_parameter("wvot", [C, C], FR, isOutput=False)
    bq_e = nc.declare_dram_parameter("bq", [C], DT, isOutput=False)
    bk_e = nc.declare_dram_parameter("bk", [C], DT, isOutput=False)
    bop_e = nc.declare_dram_parameter("bop", [C], FR, isOutput=False)
    ones_r_e = nc.declare_dram_parameter("ones_r", [128], FR, isOutput=False)
    out_e = nc.declare_dram_parameter("outT", [C, NQ], DT, isOutput=True)

    with tile.TileContext(nc) as tc:
        with (
            tc.tile_pool(name="kt", bufs=1) as kt_pool,
            tc.tile_pool(name="vv", bufs=1) as vv_pool,
            tc.tile_pool(name="wq", bufs=1) as wq_pool,
            tc.tile_pool(name="consts", bufs=1) as c_pool,
        ):
            # ---- persistent tiles (phase-2-only DMAs emitted late so they
            # don't delay the first phase-1 matmul) ----
            kt_sb = [kt_pool.tile([128, HW], FR, tag=f"k{i}", name=f"k{i}") for i in range(CK)]
            vw_sb = [vv_pool.tile([128, C], FR, tag=f"v{i}", name=f"v{i}") for i in range(MT)]
            wq_sb = [wq_pool.tile([128, C], FR, tag=f"wq{i}", name=f"wq{i}") for i in range(CK)]

            bq_t = c_pool.tile([128, CK], DT, tag="bq", name="bq_t")
            bk_t = c_pool.tile([128, CK], DT, tag="bk", name="bk_t")
            bop_row = c_pool.tile([1, C], FR, tag="bop", name="bop_row")
            ones_col_r = c_pool.tile([128, 1], FR, tag="onescr", name="ones_col_r")
            for t in range(CK):
                nc.sync.dma_start(bk_t[:, t:t + 1], bk_e[t * 128:(t + 1) * 128])
            nc.sync.dma_start(ones_col_r[:, 0:1], ones_r_e[:])

            # ---- phase 1: kT (c,m) and VW (m,c) projections ----
            with (
                tc.tile_pool(name="wkv", bufs=1) as wkv_pool,
                tc.tile_pool(name="tcc", bufs=2) as tcc_pool,
                tc.tile_pool(name="ps1", bufs=2, space="PSUM") as ps1,
            ):
                wk_sb = [wkv_pool.tile([128, C], FR, tag=f"wk{i}", name=f"wk{i}") for i in range(CK)]
                wv_sb = [wkv_pool.tile([128, C], FR, tag=f"wv{i}", name=f"wv{i}") for i in range(CK)]
                for i in range(CK):
                    nc.sync.dma_start(wk_sb[i][:], wkt_e[i * 128:(i + 1) * 128, :])
                for i in range(CK):
                    nc.sync.dma_start(wv_sb[i][:], wvot_e[i * 128:(i + 1) * 128, :])

                for j in range(HW // 512):
                    tcs = [tcc_pool.tile([128, 512], FR, tag=f"tc{ci}", name=f"tc{ci}") for ci in range(CK)]
                    for ci in range(CK):
                        nc.gpsimd.dma_start(
                            tcs[ci][:], xt_e[ci * 128:(ci + 1) * 128, j * 512:(j + 1) * 512]
                        )
                    # kT token-chunk j, all four output-channel chunks
                    for co in range(CK):
                        pk = ps1.tile([128, 512], DT, tag="pk", name="pk")
                        for ci in range(CK):
                            nc.tensor.matmul(
                                pk[:], wk_sb[ci][:, co * 128:(co + 1) * 128],
                                tcs[ci][:], start=(ci == 0), stop=(ci == CK - 1),
                            )
                        nc.scalar.activation(
                            kt_sb[co][:, j * 512:(j + 1) * 512], pk[:], AFT.Identity,
                            bias=bk_t[:, co:co + 1],
                        )
                    # VW m-tiles 4j..4j+3 (no bias: folded into bo')
                    for ml in range(4):
                        pv = ps1.tile([128, 512], DT, tag="pv", name="pv")
                        for ci in range(CK):
                            nc.tensor.matmul(
                                pv[:], tcs[ci][:, ml * 128:(ml + 1) * 128],
                                wv_sb[ci][:], start=(ci == 0), stop=(ci == CK - 1),
                            )
                        nc.vector.tensor_copy(vw_sb[4 * j + ml][:], pv[:])

            # phase-2 weights/consts arrive while phase-1 compute runs
            for i in range(CK):
                nc.sync.dma_start(wq_sb[i][:], wqt_e[i * 128:(i + 1) * 128, :])
            for t in range(CK):
                nc.sync.dma_start(bq_t[:, t:t + 1], bq_e[t * 128:(t + 1) * 128])
            nc.sync.dma_start(bop_row[0:1, :], bop_e[:])

            # ---- phase 2: attention per 512-token q-chunk ----
            with (
                tc.tile_pool(name="xqp", bufs=1) as xq_pool,
                tc.tile_pool(name="qcp", bufs=1) as qc_pool,
                tc.tile_pool(name="pexp", bufs=3) as pe_pool,
                tc.tile_pool(name="accp", bufs=2) as acc_pool,
                tc.tile_pool(name="rsp", bufs=2) as rs_pool,
                tc.tile_pool(name="outp", bufs=3) as out_pool,
                tc.tile_pool(name="smallp", bufs=2) as small_pool,
                tc.tile_pool(name="ps2", bufs=3, space="PSUM") as ps2,
                tc.tile_pool(name="psot", bufs=1, space="PSUM") as psot,
                tc.tile_pool(name="psrs", bufs=1, space="PSUM") as psrs,
            ):
                for nb in range(NB):
                    xqs = [xq_pool.tile([128, 512], FR, tag=f"xq{ci}", name=f"xq{ci}") for ci in range(CK)]
                    for ci in range(CK):
                        nc.gpsimd.dma_start(
                            xqs[ci][:], xq_e[ci * 128:(ci + 1) * 128, nb * 512:(nb + 1) * 512]
                        )
                    # qT chunk (c, 512)
                    qcs = []
                    for co in range(CK):
                        pq = ps2.tile([128, 512], DT, tag="st", name="st")
                        for ci in range(CK):
                            nc.tensor.matmul(
                                pq[:], wq_sb[ci][:, co * 128:(co + 1) * 128],
                                xqs[ci][:], start=(ci == 0), stop=(ci == CK - 1),
                            )
                        qc = qc_pool.tile([128, 512], FR, tag=f"qc{co}", name=f"qc{co}")
                        nc.scalar.activation(qc[:], pq[:], AFT.Identity, bias=bq_t[:, co:co + 1])
                        qcs.append(qc)

                    acc = acc_pool.tile([128, 512], FR, tag="acc", name="acc")
                    ots = [psot.tile([128, 512], DT, tag=f"ot{co}", name=f"ot{co}") for co in range(CK)]
                    for mt in range(MT):
                        st = ps2.tile([128, 512], DT, tag="st", name="st")
                        for ci in range(CK):
                            nc.tensor.matmul(
                                st[:], kt_sb[ci][:, mt * 128:(mt + 1) * 128],
                                qcs[ci][:], start=(ci == 0), stop=(ci == CK - 1),
                            )
                        pexp = pe_pool.tile([128, 512], FR, tag="pe", name="pexp")
                        nc.scalar.activation(pexp[:], st[:], AFT.Exp, scale=SCALE)
                        if mt == 0:
                            nc.vector.tensor_copy(acc[:], pexp[:].bitcast(DT))
                        else:
                            nc.vector.tensor_add(acc[:], acc[:].bitcast(DT), pexp[:].bitcast(DT))
                        for co in range(CK):
                            nc.tensor.matmul(
                                ots[co][:], vw_sb[mt][:, co * 128:(co + 1) * 128],
                                pexp[:],
                                start=(mt == 0), stop=False, skip_group_check=True,
                            )

                    # rowsum via one f32r ones-matmul; reciprocal row;
                    # broadcast to 128 partitions with a 0-stride DMA
                    rs = psrs.tile([1, 512], DT, tag="rs", name="rs")
                    nc.tensor.matmul(rs[:], ones_col_r[:, 0:1], acc[:], start=True, stop=True)
                    rs_row = small_pool.tile([1, 512], FR, tag="rsrow", name="rs_row")
                    nc.scalar.activation(rs_row[:], rs[:], AFT.Copy)
                    rinv_row = small_pool.tile([1, 512], DT, tag="rinvrow", name="rinv_row")
                    nc.vector.reciprocal(rinv_row[:], rs[:])
                    rinv_bc = rs_pool.tile([128, 512], DT, tag="rinvbc", name="rinv_bc")
                    _src = rinv_row[0:1, :]
                    _p = _src.ap[0][0]
                    _bc = type(_src)(_src.tensor, _src.offset, [[_p, 1], [0, 128], [1, 512]])
                    nc.sync.dma_start(rinv_bc[:, :], _bc)

                    # exact bias (bo'[c] x rowsum[n]) closes each accumulation
                    # group; then normalize on PSUM->SBUF evac and store
                    for co in range(CK):
                        nc.tensor.matmul(
                            ots[co][:], bop_row[0:1, co * 128:(co + 1) * 128],
                            rs_row[0:1, :], start=False, stop=True, skip_group_check=True,
                        )
                        oc = out_pool.tile([128, 512], DT, tag="oc", name="oc")
                        nc.vector.tensor_mul(oc[:], ots[co][:], rinv_bc[:])
                        nc.sync.dma_start(
                            out_e[co * 128:(co + 1) * 128, nb * 512:(nb + 1) * 512], oc[:]
                        )

    nc.compile()
    return nc


def _get_compiled():
    global _compiled
    if _compiled is None:
        _compiled = _build()
    return _compiled


def kernel(**inputs):
    x = np.ascontiguousarray(np.asarray(inputs["x"], dtype=np.float32))
    wq = np.asarray(inputs["Wq"], dtype=np.float32)
    wk = np.asarray(inputs["Wk"], dtype=np.float32)
    wv = np.asarray(inputs["Wv"], dtype=np.float32)
    wo = np.asarray(inputs["Wo"], dtype=np.float32)
    bq = np.ascontiguousarray(np.asarray(inputs["bq"], dtype=np.float32))
    bk = np.ascontiguousarray(np.asarray(inputs["bk"], dtype=np.float32))
    bv = np.asarray(inputs["bv"], dtype=np.float32)
    bo = np.asarray(inputs["bo"], dtype=np.float32)

    wqt = np.ascontiguousarray(wq.T)
    wkt = np.ascontiguousarray(wk.T)
    wvot = np.ascontiguousarray((wo @ wv).T)     # fused V/output projection
    bop = np.ascontiguousarray(wo @ bv + bo)

    xb = x.reshape(B, C, HW)
    in_maps = []
    for core in range(N_CORES):
        bi, h = core // 2, core % 2
        in_maps.append({
            "xt": np.ascontiguousarray(xb[bi]),
            "xq": np.ascontiguousarray(xb[bi][:, h * NQ:(h + 1) * NQ]),
            "wqt": wqt, "wkt": wkt, "wvot": wvot,
            "bq": bq, "bk": bk, "bop": bop, "ones_r": _ONES,
        })

    nc = _get_compiled()
    res = run_bass_kernel_spmd(nc, in_maps, core_ids=list(range(N_CORES)))

    out = np.empty((B, HW, C), dtype=np.float32)
    for core in range(N_CORES):
        bi, h = core // 2, core % 2
        out[bi, h * NQ:(h + 1) * NQ, :] = res.results[core]["outT"].T
    return out.reshape(B, C, 64, 64)


# revision 16
# speedup vs baseline: 1.0940x; 1.0940x over previous
"""Trainium2 Bass kernel for nn_Attention_57080115364834.

Reference computation (B=4, C=512, H=W=64, N=H*W=4096 tokens):
    t = x.reshape(b, c, n).swapaxes(1, 2)          # (b, n, c)
    q, k, v = t@Wq.T+bq, t@Wk.T+bk, t@Wv.T+bv
    attn = softmax(q @ k.T / sqrt(c))              # (b, n, n)
    out = (attn @ v) @ Wo.T + bo                   # (b, n, c)
    return out.reshape(b, c, h, w)                 # raw view, no permute

Sharding: 8 cores = 4 batches x 2 query-halves. Each core holds the full
x[b] (C x N, which is exactly t.T - the natural layout for Trainium
matmuls) so it computes its batch's full K^T (c,n) and VW (n,c) locally,
plus Q^T for its 2048-token half. No collectives.

Host-side algebra folds both post-attention linear steps away:
  - softmax rows sum to 1  =>  attn @ (v+bv) == attn@v + bv, so the v
    bias becomes an output bias  bo' = Wo @ bv + bo.
  - (attn @ v) @ Wo.T == attn @ (v @ Wo.T) == attn @ (t @ (Wo@Wv).T),
    so with Wvo = Wo@Wv precomputed on host, the VW projection directly
    produces final-channel values and no device-side output projection
    is needed.
The kernel returns outT (c, n) per core; the host transposes during
unsharding (a pure layout move).

Per-core dataflow (main matmuls in float32r = full-rate fp32):
  kT[c,m]   = Wk @ tC + bk   (lhsT=WkT chunk, rhs=tC chunk; bias on ACT evac)
  VW[m,c]   = tC.T @ WvoT    (lhsT=tC chunk,  rhs=WvoT)
  qT[c,n]   = Wq @ tCq + bq  per 512-token n-chunk
  ST[m,n]   = kT.T-chunks @ qT       (scores, transposed)
  P[m,n]    = exp(ST/sqrt(c))        ScalarE, no max-subtract (|scores|<~2)
  acc[m%128,n] += P                  DVE accumulate (for rowsum)
  OT[c,n]  += VW-chunk.T @ P         (PSUM-accumulated over m-tiles)
  OT[c,n]  += bo'[c-chunk] x rowsum[n]   (K=1 matmul; exact bias)
  rowsum_bc = gpsimd partition_all_reduce(acc); rinv_bc = 1/rowsum_bc
  outT[c,n] = OT * rinv_bc           (DVE, PSUM->SBUF) -> DMA
"""

import sys

for _p in ("/opt/trn_rl_repo", "/root/.axon_site/_ro/trn_rl_repo"):
    if _p not in sys.path:
        sys.path.append(_p)

import numpy as np

import concourse.bacc as bacc
import concourse.bass_isa as bass_isa
import concourse.mybir as mybir
import concourse.tile as tile
from concourse.bass_utils import run_bass_kernel_spmd

DT = mybir.dt.float32
FR = mybir.dt.float32r
AFT = mybir.ActivationFunctionType

B, C, HW = 4, 512, 4096          # batch, channels, tokens per batch
NQ = HW // 2                     # q tokens per core (2048)
CK = C // 128                    # contraction chunks (4)
MT = HW // 128                   # key/value tiles (32)
NB = NQ // 512                   # q-chunks per core (4)
SCALE = 1.0 / float(np.sqrt(C))
N_CORES = 8

_compiled = None
_ONES = np.ones(128, dtype=np.float32)


def _build():
    nc = bacc.Bacc("TRN2", target_bir_lowering=False)

    xt_e = nc.declare_dram_parameter("xt", [C, HW], FR, isOutput=False)
    xq_e = nc.declare_dram_parameter("xq", [C, NQ], FR, isOutput=False)
    wqt_e = nc.declare_dram_parameter("wqt", [C, C], FR, isOutput=False)
    wkt_e = nc.declare_dram_parameter("wkt", [C, C], FR, isOutput=False)
    wvot_e = nc.declare_dram_parameter("wvot", [C, C], FR, isOutput=False)
    bq_e = nc.declare_dram_parameter("bq", [C], DT, isOutput=False)
    bk_e = nc.declare_dram_parameter("bk", [C], DT, isOutput=False)
    bop_e = nc.declare_dram_parameter("bop", [C], FR, isOutput=False)
    ones_r_e = nc.declare_dram_parameter("ones_r", [128], FR, isOutput=False)
    out_e = nc.declare_dram_parameter("outT", [C, NQ], DT, isOutput=True)

    with tile.TileContext(nc) as tc:
        with (
            tc.tile_pool(name="kt", bufs=1) as kt_pool,
            tc.tile_pool(name="vv", bufs=1) as vv_pool,
            tc.tile_pool(name="wq", bufs=1) as wq_pool,
            tc.tile_pool(name="consts", bufs=1) as c_pool,
        ):
            # ---- persistent tiles (phase-2-only DMAs emitted late so they
            # don't delay the first phase-1 matmul) ----
            kt_sb = [kt_pool.tile([128, HW], FR, tag=f"k{i}", name=f"k{i}") for i in range(CK)]
            vw_sb = [vv_pool.tile([128, C], FR, tag=f"v{i}", name=f"v{i}") for i in range(MT)]
            wq_sb = [wq_pool.tile([128, C], FR, tag=f"wq{i}", name=f"wq{i}") for i in range(CK)]

            bq_t = c_pool.tile([128, CK], DT, tag="bq", name="bq_t")
            bk_t = c_pool.tile([128, CK], DT, tag="bk", name="bk_t")
            bop_row = c_pool.tile([1, C], FR, tag="bop", name="bop_row")
            ones_col_r = c_pool.tile([128, 1], FR, tag="onescr", name="ones_col_r")
            ones_row_r = c_pool.tile([1, 128], FR, tag="onesrr", name="ones_row_r")
            for t in range(CK):
                nc.sync.dma_start(bk_t[:, t:t + 1], bk_e[t * 128:(t + 1) * 128])
            nc.sync.dma_start(ones_col_r[:, 0:1], ones_r_e[:])
            nc.sync.dma_start(ones_row_r[0:1, :], ones_r_e[:])

            # ---- phase 1: kT (c,m) and VW (m,c) projections ----
            with (
                tc.tile_pool(name="wkv", bufs=1) as wkv_pool,
                tc.tile_pool(name="tcc", bufs=2) as tcc_pool,
                tc.tile_pool(name="ps1", bufs=2, space="PSUM") as ps1,
            ):
                wk_sb = [wkv_pool.tile([128, C], FR, tag=f"wk{i}", name=f"wk{i}") for i in range(CK)]
                wv_sb = [wkv_pool.tile([128, C], FR, tag=f"wv{i}", name=f"wv{i}") for i in range(CK)]
                for i in range(CK):
                    nc.sync.dma_start(wk_sb[i][:], wkt_e[i * 128:(i + 1) * 128, :])
                for i in range(CK):
                    nc.sync.dma_start(wv_sb[i][:], wvot_e[i * 128:(i + 1) * 128, :])

                for j in range(HW // 512):
                    tcs = [tcc_pool.tile([128, 512], FR, tag=f"tc{ci}", name=f"tc{ci}") for ci in range(CK)]
                    for ci in range(CK):
                        nc.gpsimd.dma_start(
                            tcs[ci][:], xt_e[ci * 128:(ci + 1) * 128, j * 512:(j + 1) * 512]
                        )
                    # kT token-chunk j, all four output-channel chunks
                    for co in range(CK):
                        pk = ps1.tile([128, 512], DT, tag="pk", name="pk")
                        for ci in range(CK):
                            nc.tensor.matmul(
                                pk[:], wk_sb[ci][:, co * 128:(co + 1) * 128],
                                tcs[ci][:], start=(ci == 0), stop=(ci == CK - 1),
                            )
                        nc.scalar.activation(
                            kt_sb[co][:, j * 512:(j + 1) * 512], pk[:], AFT.Identity,
                            bias=bk_t[:, co:co + 1],
                        )
                    # VW m-tiles 4j..4j+3 (no bias: folded into bo')
                    for ml in range(4):
                        pv = ps1.tile([128, 512], DT, tag="pv", name="pv")
                        for ci in range(CK):
                            nc.tensor.matmul(
                                pv[:], tcs[ci][:, ml * 128:(ml + 1) * 128],
                                wv_sb[ci][:], start=(ci == 0), stop=(ci == CK - 1),
                            )
                        nc.vector.tensor_copy(vw_sb[4 * j + ml][:], pv[:])

            # phase-2 weights/consts arrive while phase-1 compute runs
            for i in range(CK):
                nc.sync.dma_start(wq_sb[i][:], wqt_e[i * 128:(i + 1) * 128, :])
            for t in range(CK):
                nc.sync.dma_start(bq_t[:, t:t + 1], bq_e[t * 128:(t + 1) * 128])
            nc.sync.dma_start(bop_row[0:1, :], bop_e[:])

            # ---- phase 2: attention per 512-token q-chunk ----
            with (
                tc.tile_pool(name="xqp", bufs=1) as xq_pool,
                tc.tile_pool(name="qcp", bufs=1) as qc_pool,
                tc.tile_pool(name="pexp", bufs=3) as pe_pool,
                tc.tile_pool(name="accp", bufs=2) as acc_pool,
                tc.tile_pool(name="rsp", bufs=2) as rs_pool,
                tc.tile_pool(name="outp", bufs=3) as out_pool,
                tc.tile_pool(name="smallp", bufs=2) as small_pool,
                tc.tile_pool(name="ps2", bufs=3, space="PSUM") as ps2,
                tc.tile_pool(name="psot", bufs=1, space="PSUM") as psot,
                tc.tile_pool(name="psrs", bufs=1, space="PSUM") as psrs,
            ):
                for nb in range(NB):
                    xqs = [xq_pool.tile([128, 512], FR, tag=f"xq{ci}", name=f"xq{ci}") for ci in range(CK)]
                    for ci in range(CK):
                        nc.gpsimd.dma_start(
                            xqs[ci][:], xq_e[ci * 128:(ci + 1) * 128, nb * 512:(nb + 1) * 512]
                        )
                    # qT chunk (c, 512)
                    qcs = []
                    for co in range(CK):
                        pq = ps2.tile([128, 512], DT, tag="st", name="st")
                        for ci in range(CK):
                            nc.tensor.matmul(
                                pq[:], wq_sb[ci][:, co * 128:(co + 1) * 128],
                                xqs[ci][:], start=(ci == 0), stop=(ci == CK - 1),
                            )
                        qc = qc_pool.tile([128, 512], FR, tag=f"qc{co}", name=f"qc{co}")
                        nc.scalar.activation(qc[:], pq[:], AFT.Identity, bias=bq_t[:, co:co + 1])
                        qcs.append(qc)

                    acc = acc_pool.tile([128, 512], FR, tag="acc", name="acc")
                    ots = [psot.tile([128, 512], DT, tag=f"ot{co}", name=f"ot{co}") for co in range(CK)]
                    for mt in range(MT):
                        st = ps2.tile([128, 512], DT, tag="st", name="st")
                        for ci in range(CK):
                            nc.tensor.matmul(
                                st[:], kt_sb[ci][:, mt * 128:(mt + 1) * 128],
                                qcs[ci][:], start=(ci == 0), stop=(ci == CK - 1),
                            )
                        pexp = pe_pool.tile([128, 512], FR, tag="pe", name="pexp")
                        nc.scalar.activation(pexp[:], st[:], AFT.Exp, scale=SCALE)
                        if mt == 0:
                            nc.vector.tensor_copy(acc[:], pexp[:].bitcast(DT))
                        else:
                            nc.vector.tensor_add(acc[:], acc[:].bitcast(DT), pexp[:].bitcast(DT))
                        for co in range(CK):
                            nc.tensor.matmul(
                                ots[co][:], vw_sb[mt][:, co * 128:(co + 1) * 128],
                                pexp[:],
                                start=(mt == 0), stop=False, skip_group_check=True,
                            )

                    # rowsum via one f32r ones-matmul; reciprocal row;
                    # broadcast to 128 partitions with a 0-stride DMA
                    rs = psrs.tile([1, 512], DT, tag="rs", name="rs")
                    nc.tensor.matmul(rs[:], ones_col_r[:, 0:1], acc[:], start=True, stop=True)
                    rs_row = small_pool.tile([1, 512], FR, tag="rsrow", name="rs_row")
                    nc.scalar.activation(rs_row[:], rs[:], AFT.Copy)
                    rinv_row = small_pool.tile([1, 512], FR, tag="rinvrow", name="rinv_row")
                    with nc.allow_low_precision(reason="f32r bits == f32 bits; PE rounds on read"):
                        nc.vector.reciprocal(rinv_row[:], rs[:])
                    rbc_ps = psrs.tile([128, 512], DT, tag="rs", name="rbc_ps")
                    nc.tensor.matmul(rbc_ps[:], ones_row_r[0:1, :], rinv_row[0:1, :],
                                     start=True, stop=True)
                    rinv_bc = rs_pool.tile([128, 512], DT, tag="rinvbc", name="rinv_bc")
                    nc.vector.tensor_copy(rinv_bc[:], rbc_ps[:])

                    # exact bias (bo'[c] x rowsum[n]) closes each accumulation
                    # group; then normalize on PSUM->SBUF evac and store
                    for co in range(CK):
                        nc.tensor.matmul(
                            ots[co][:], bop_row[0:1, co * 128:(co + 1) * 128],
                            rs_row[0:1, :], start=False, stop=True, skip_group_check=True,
                        )
                        oc = out_pool.tile([128, 512], DT, tag="oc", name="oc")
                        nc.vector.tensor_mul(oc[:], ots[co][:], rinv_bc[:])
                        nc.sync.dma_start(
                            out_e[co * 128:(co + 1) * 128, nb * 512:(nb + 1) * 512], oc[:]
                        )

    nc.compile()
    return nc


def _get_compiled():
    global _compiled
    if _compiled is None:
        _compiled = _build()
    return _compiled


def kernel(**inputs):
    x = np.ascontiguousarray(np.asarray(inputs["x"], dtype=np.float32))
    wq = np.asarray(inputs["Wq"], dtype=np.float32)
    wk = np.asarray(inputs["Wk"], dtype=np.float32)
    wv = np.asarray(inputs["Wv"], dtype=np.float32)
    wo = np.asarray(inputs["Wo"], dtype=np.float32)
    bq = np.ascontiguousarray(np.asarray(inputs["bq"], dtype=np.float32))
    bk = np.ascontiguousarray(np.asarray(inputs["bk"], dtype=np.float32))
    bv = np.asarray(inputs["bv"], dtype=np.float32)
    bo = np.asarray(inputs["bo"], dtype=np.float32)

    wqt = np.ascontiguousarray(wq.T)
    wkt = np.ascontiguousarray(wk.T)
    wvot = np.ascontiguousarray((wo @ wv).T)     # fused V/output projection
    bop = np.ascontiguousarray(wo @ bv + bo)

    xb = x.reshape(B, C, HW)
    in_maps = []
    for core in range(N_CORES):
        bi, h = core // 2, core % 2
        in_maps.append({
            "xt": np.ascontiguousarray(xb[bi]),
            "xq": np.ascontiguousarray(xb[bi][:, h * NQ:(h + 1) * NQ]),
            "wqt": wqt, "wkt": wkt, "wvot": wvot,
            "bq": bq, "bk": bk, "bop": bop, "ones_r": _ONES,
        })

    nc = _get_compiled()
    res = run_bass_kernel_spmd(nc, in_maps, core_ids=list(range(N_CORES)))

    out = np.empty((B, HW, C), dtype=np.float32)
    for core in range(N_CORES):
        bi, h = core // 2, core % 2
        out[bi, h * NQ:(h + 1) * NQ, :] = res.results[core]["outT"].T
    return out.reshape(B, C, 64, 64)


# revision 17
# speedup vs baseline: 1.1738x; 1.0730x over previous
"""Trainium2 Bass kernel for nn_Attention_57080115364834.

Reference computation (B=4, C=512, H=W=64, N=H*W=4096 tokens):
    t = x.reshape(b, c, n).swapaxes(1, 2)          # (b, n, c)
    q, k, v = t@Wq.T+bq, t@Wk.T+bk, t@Wv.T+bv
    attn = softmax(q @ k.T / sqrt(c))              # (b, n, n)
    out = (attn @ v) @ Wo.T + bo                   # (b, n, c)
    return out.reshape(b, c, h, w)                 # raw view, no permute

Sharding: 8 cores = 4 batches x 2 query-halves. Each core holds the full
x[b] (C x N, which is exactly t.T - the natural layout for Trainium
matmuls) so it computes its batch's full K^T (c,n) and VW (n,c) locally,
plus Q^T for its 2048-token half. No collectives.

Host-side algebra folds both post-attention linear steps away:
  - softmax rows sum to 1  =>  attn @ (v+bv) == attn@v + bv, so the v
    bias becomes an output bias  bo' = Wo @ bv + bo.
  - (attn @ v) @ Wo.T == attn @ (v @ Wo.T) == attn @ (t @ (Wo@Wv).T),
    so with Wvo = Wo@Wv precomputed on host, the VW projection directly
    produces final-channel values and no device-side output projection
    is needed.
The kernel returns outT (c, n) per core; the host transposes during
unsharding (a pure layout move).

Per-core dataflow (main matmuls in float32r = full-rate fp32):
  kT[c,m]   = Wk @ tC + bk   (lhsT=WkT chunk, rhs=tC chunk; bias on ACT evac)
  VW[m,c]   = tC.T @ WvoT    (lhsT=tC chunk,  rhs=WvoT)
  qT[c,n]   = Wq @ tCq + bq  per 512-token n-chunk
  ST[m,n]   = kT.T-chunks @ qT       (scores, transposed)
  P[m,n]    = exp(ST/sqrt(c))        ScalarE, no max-subtract (|scores|<~2)
  acc[m%128,n] += P                  DVE accumulate (for rowsum)
  OT[c,n]  += VW-chunk.T @ P         (PSUM-accumulated over m-tiles)
  OT[c,n]  += bo'[c-chunk] x rowsum[n]   (K=1 matmul; exact bias)
  rowsum_bc = gpsimd partition_all_reduce(acc); rinv_bc = 1/rowsum_bc
  outT[c,n] = OT * rinv_bc           (DVE, PSUM->SBUF) -> DMA
"""

import sys

for _p in ("/opt/trn_rl_repo", "/root/.axon_site/_ro/trn_rl_repo"):
    if _p not in sys.path:
        sys.path.append(_p)

import numpy as np
import ml_dtypes

import concourse.bacc as bacc
import concourse.bass_isa as bass_isa
import concourse.mybir as mybir
import concourse.tile as tile
from concourse.bass_utils import run_bass_kernel_spmd

DT = mybir.dt.float32
FR = mybir.dt.float32r
BF = mybir.dt.bfloat16
AFT = mybir.ActivationFunctionType

B, C, HW = 4, 512, 4096          # batch, channels, tokens per batch
NQ = HW // 2                     # q tokens per core (2048)
CK = C // 128                    # contraction chunks (4)
MT = HW // 128                   # key/value tiles (32)
NB = NQ // 512                   # q-chunks per core (4)
SCALE = 1.0 / float(np.sqrt(C))
N_CORES = 8

_compiled = None
_ONES = np.ones(128, dtype=np.float32)


def _build():
    nc = bacc.Bacc("TRN2", target_bir_lowering=False)

    xt_e = nc.declare_dram_parameter("xt", [C, HW], BF, isOutput=False)
    xq_e = nc.declare_dram_parameter("xq", [C, NQ], BF, isOutput=False)
    wqt_e = nc.declare_dram_parameter("wqt", [C, C], BF, isOutput=False)
    wkt_e = nc.declare_dram_parameter("wkt", [C, C], BF, isOutput=False)
    wvot_e = nc.declare_dram_parameter("wvot", [C, C], BF, isOutput=False)
    bq_e = nc.declare_dram_parameter("bq", [C], DT, isOutput=False)
    bk_e = nc.declare_dram_parameter("bk", [C], DT, isOutput=False)
    bop_e = nc.declare_dram_parameter("bop", [C], BF, isOutput=False)
    ones_r_e = nc.declare_dram_parameter("ones_r", [128], DT, isOutput=False)
    out_e = nc.declare_dram_parameter("outT", [C, NQ], DT, isOutput=True)

    with tile.TileContext(nc) as tc:
        with (
            tc.tile_pool(name="kt", bufs=1) as kt_pool,
            tc.tile_pool(name="vv", bufs=1) as vv_pool,
            tc.tile_pool(name="wq", bufs=1) as wq_pool,
            tc.tile_pool(name="consts", bufs=1) as c_pool,
        ):
            # ---- persistent tiles (phase-2-only DMAs emitted late so they
            # don't delay the first phase-1 matmul) ----
            kt_sb = [kt_pool.tile([128, HW], BF, tag=f"k{i}", name=f"k{i}") for i in range(CK)]
            vw_sb = [vv_pool.tile([128, C], BF, tag=f"v{i}", name=f"v{i}") for i in range(MT)]
            wq_sb = [wq_pool.tile([128, C], BF, tag=f"wq{i}", name=f"wq{i}") for i in range(CK)]

            bq_t = c_pool.tile([128, CK], DT, tag="bq", name="bq_t")
            bk_t = c_pool.tile([128, CK], DT, tag="bk", name="bk_t")
            bop_row = c_pool.tile([1, C], BF, tag="bop", name="bop_row")
            ones_col_r = c_pool.tile([128, 1], DT, tag="onescr", name="ones_col_r")
            ones_row_r = c_pool.tile([1, 128], DT, tag="onesrr", name="ones_row_r")
            for t in range(CK):
                nc.sync.dma_start(bk_t[:, t:t + 1], bk_e[t * 128:(t + 1) * 128])
            nc.sync.dma_start(ones_col_r[:, 0:1], ones_r_e[:])
            nc.sync.dma_start(ones_row_r[0:1, :], ones_r_e[:])

            # ---- phase 1: kT (c,m) and VW (m,c) projections ----
            with (
                tc.tile_pool(name="wkv", bufs=1) as wkv_pool,
                tc.tile_pool(name="tcc", bufs=2) as tcc_pool,
                tc.tile_pool(name="ps1", bufs=2, space="PSUM") as ps1,
            ):
                wk_sb = [wkv_pool.tile([128, C], BF, tag=f"wk{i}", name=f"wk{i}") for i in range(CK)]
                wv_sb = [wkv_pool.tile([128, C], BF, tag=f"wv{i}", name=f"wv{i}") for i in range(CK)]
                for i in range(CK):
                    nc.sync.dma_start(wk_sb[i][:], wkt_e[i * 128:(i + 1) * 128, :])
                for i in range(CK):
                    nc.sync.dma_start(wv_sb[i][:], wvot_e[i * 128:(i + 1) * 128, :])

                for j in range(HW // 512):
                    tcs = [tcc_pool.tile([128, 512], BF, tag=f"tc{ci}", name=f"tc{ci}") for ci in range(CK)]
                    for ci in range(CK):
                        nc.gpsimd.dma_start(
                            tcs[ci][:], xt_e[ci * 128:(ci + 1) * 128, j * 512:(j + 1) * 512]
                        )
                    # kT token-chunk j, all four output-channel chunks
                    for co in range(CK):
                        pk = ps1.tile([128, 512], DT, tag="pk", name="pk")
                        for ci in range(CK):
                            nc.tensor.matmul(
                                pk[:], wk_sb[ci][:, co * 128:(co + 1) * 128],
                                tcs[ci][:], start=(ci == 0), stop=(ci == CK - 1),
                            )
                        nc.scalar.activation(
                            kt_sb[co][:, j * 512:(j + 1) * 512], pk[:], AFT.Identity,
                            bias=bk_t[:, co:co + 1],
                        )
                    # VW m-tiles 4j..4j+3 (no bias: folded into bo')
                    for ml in range(4):
                        pv = ps1.tile([128, 512], DT, tag="pv", name="pv")
                        for ci in range(CK):
                            nc.tensor.matmul(
                                pv[:], tcs[ci][:, ml * 128:(ml + 1) * 128],
                                wv_sb[ci][:], start=(ci == 0), stop=(ci == CK - 1),
                            )
                        nc.vector.tensor_copy(vw_sb[4 * j + ml][:], pv[:])

            # phase-2 weights/consts arrive while phase-1 compute runs
            for i in range(CK):
                nc.sync.dma_start(wq_sb[i][:], wqt_e[i * 128:(i + 1) * 128, :])
            for t in range(CK):
                nc.sync.dma_start(bq_t[:, t:t + 1], bq_e[t * 128:(t + 1) * 128])
            nc.sync.dma_start(bop_row[0:1, :], bop_e[:])

            # ---- phase 2: attention per 512-token q-chunk ----
            with (
                tc.tile_pool(name="xqp", bufs=1) as xq_pool,
                tc.tile_pool(name="qcp", bufs=1) as qc_pool,
                tc.tile_pool(name="pexp", bufs=3) as pe_pool,
                tc.tile_pool(name="accp", bufs=2) as acc_pool,
                tc.tile_pool(name="rsp", bufs=2) as rs_pool,
                tc.tile_pool(name="outp", bufs=3) as out_pool,
                tc.tile_pool(name="smallp", bufs=2) as small_pool,
                tc.tile_pool(name="ps2", bufs=3, space="PSUM") as ps2,
                tc.tile_pool(name="psot", bufs=1, space="PSUM") as psot,
                tc.tile_pool(name="psrs", bufs=1, space="PSUM") as psrs,
            ):
                for nb in range(NB):
                    xqs = [xq_pool.tile([128, 512], BF, tag=f"xq{ci}", name=f"xq{ci}") for ci in range(CK)]
                    for ci in range(CK):
                        nc.gpsimd.dma_start(
                            xqs[ci][:], xq_e[ci * 128:(ci + 1) * 128, nb * 512:(nb + 1) * 512]
                        )
                    # qT chunk (c, 512)
                    qcs = []
                    for co in range(CK):
                        pq = ps2.tile([128, 512], DT, tag="st", name="st")
                        for ci in range(CK):
                            nc.tensor.matmul(
                                pq[:], wq_sb[ci][:, co * 128:(co + 1) * 128],
                                xqs[ci][:], start=(ci == 0), stop=(ci == CK - 1),
                            )
                        qc = qc_pool.tile([128, 512], BF, tag=f"qc{co}", name=f"qc{co}")
                        nc.scalar.activation(qc[:], pq[:], AFT.Identity, bias=bq_t[:, co:co + 1])
                        qcs.append(qc)

                    acc = acc_pool.tile([128, 512], DT, tag="acc", name="acc")
                    ots = [psot.tile([128, 512], DT, tag=f"ot{co}", name=f"ot{co}") for co in range(CK)]
                    for mt in range(MT):
                        st = ps2.tile([128, 512], DT, tag="st", name="st")
                        for ci in range(CK):
                            nc.tensor.matmul(
                                st[:], kt_sb[ci][:, mt * 128:(mt + 1) * 128],
                                qcs[ci][:], start=(ci == 0), stop=(ci == CK - 1),
                            )
                        pexp = pe_pool.tile([128, 512], BF, tag="pe", name="pexp")
                        nc.scalar.activation(pexp[:], st[:], AFT.Exp, scale=SCALE)
                        if mt == 0:
                            nc.vector.tensor_copy(acc[:], pexp[:])
                        else:
                            nc.vector.tensor_add(acc[:], acc[:], pexp[:])
                        for co in range(CK):
                            nc.tensor.matmul(
                                ots[co][:], vw_sb[mt][:, co * 128:(co + 1) * 128],
                                pexp[:],
                                start=(mt == 0), stop=False, skip_group_check=True,
                            )

                    # rowsum via one f32r ones-matmul; reciprocal row;
                    # broadcast to 128 partitions with a 0-stride DMA
                    rs = psrs.tile([1, 512], DT, tag="rs", name="rs")
                    nc.tensor.matmul(rs[:], ones_col_r[:, 0:1], acc[:], start=True, stop=True)
                    rs_row = small_pool.tile([1, 512], BF, tag="rsrow", name="rs_row")
                    nc.scalar.activation(rs_row[:], rs[:], AFT.Copy)
                    rinv_row = small_pool.tile([1, 512], DT, tag="rinvrow", name="rinv_row")
                    nc.vector.reciprocal(rinv_row[:], rs[:])
                    rbc_ps = psrs.tile([128, 512], DT, tag="rs", name="rbc_ps")
                    nc.tensor.matmul(rbc_ps[:], ones_row_r[0:1, :], rinv_row[0:1, :],
                                     start=True, stop=True)
                    rinv_bc = rs_pool.tile([128, 512], DT, tag="rinvbc", name="rinv_bc")
                    nc.vector.tensor_copy(rinv_bc[:], rbc_ps[:])

                    # exact bias (bo'[c] x rowsum[n]) closes each accumulation
                    # group; then normalize on PSUM->SBUF evac and store
                    for co in range(CK):
                        nc.tensor.matmul(
                            ots[co][:], bop_row[0:1, co * 128:(co + 1) * 128],
                            rs_row[0:1, :], start=False, stop=True, skip_group_check=True,
                        )
                        oc = out_pool.tile([128, 512], DT, tag="oc", name="oc")
                        nc.vector.tensor_mul(oc[:], ots[co][:], rinv_bc[:])
                        nc.sync.dma_start(
                            out_e[co * 128:(co + 1) * 128, nb * 512:(nb + 1) * 512], oc[:]
                        )

    nc.compile()
    return nc


def _get_compiled():
    global _compiled
    if _compiled is None:
        _compiled = _build()
    return _compiled


def kernel(**inputs):
    x = np.ascontiguousarray(np.asarray(inputs["x"], dtype=np.float32))
    wq = np.asarray(inputs["Wq"], dtype=np.float32)
    wk = np.asarray(inputs["Wk"], dtype=np.float32)
    wv = np.asarray(inputs["Wv"], dtype=np.float32)
    wo = np.asarray(inputs["Wo"], dtype=np.float32)
    bq = np.ascontiguousarray(np.asarray(inputs["bq"], dtype=np.float32))
    bk = np.ascontiguousarray(np.asarray(inputs["bk"], dtype=np.float32))
    bv = np.asarray(inputs["bv"], dtype=np.float32)
    bo = np.asarray(inputs["bo"], dtype=np.float32)

    wqt = np.ascontiguousarray(wq.T.astype(ml_dtypes.bfloat16))
    wkt = np.ascontiguousarray(wk.T.astype(ml_dtypes.bfloat16))
    wvot = np.ascontiguousarray((wo @ wv).T.astype(ml_dtypes.bfloat16))
    bop = np.ascontiguousarray((wo @ bv + bo).astype(ml_dtypes.bfloat16))

    xb = x.reshape(B, C, HW).astype(ml_dtypes.bfloat16)
    in_maps = []
    for core in range(N_CORES):
        bi, h = core // 2, core % 2
        in_maps.append({
            "xt": np.ascontiguousarray(xb[bi]),
            "xq": np.ascontiguousarray(xb[bi][:, h * NQ:(h + 1) * NQ]),
            "wqt": wqt, "wkt": wkt, "wvot": wvot,
            "bq": bq, "bk": bk, "bop": bop, "ones_r": _ONES,
        })

    nc = _get_compiled()
    res = run_bass_kernel_spmd(nc, in_maps, core_ids=list(range(N_CORES)))

    out = np.empty((B, HW, C), dtype=np.float32)
    for core in range(N_CORES):
        bi, h = core // 2, core % 2
        out[bi, h * NQ:(h + 1) * NQ, :] = res.results[core]["outT"].T
    return out.reshape(B, C, 64, 64)
